# revision 14
# baseline (speedup 1.0000x reference)
"""ClassBalancedSupConLoss on 8 TRN2 NeuronCores (Bass/Tile) — v2.

Key change vs v1: CLASS-SKIP.  Anchors are re-permuted into class-pure
128-row tiles (leftovers form 1-2 mixed tiles).  A pure class-c tile's
denominator needs bank columns of classes != c only, so each core's
bank stream is a host-PACKED dense array of just those columns — the
own-class ~1/3 of the bank (previously exp'd and then subtracted) is
never computed.  bb (batch) columns are always fully included (positives
are part of the reference denominator), so bank+bb merge into a single
running accumulation per tile: one accum_out per PSUM block, summed on
the host.  The self term exp(invt*(s_ii-1)) is subtracted on the HOST
from the device-computed s_ii (spline-vs-np.exp difference ~2ULP is
negligible against the 2e-2 gate).

SPMD uniformity: one program for all 8 cores; all per-core variation is
in the packed DATA (which bank columns, per-call inclusion masks and
zero-pad dummy counts live host-side).  Program constants (segment
sizes, call cuts) come from the label histograms, baked at compile time.

Layout per core (wslot = weight slot: 0,1 = own tiles, 2.. = mixed
foreign tiles shared by all cores):
  stream = [slot0: bank KB | bb 2048]   (one cut-group: pure tile)
           [slot1: group m_x | group m_y | bb 2048]  (cuts at groups)
           [FS: ceil(m_cmin/8) per mixed tile]       (foreign share)
Calls = PSUM blocks (<=2048 cols), new block at every cut-group/wslot
change.  Host reduction: den_i = sum over included calls of
(accum - ndum*exp(-invt_i)) - exp(invt_i*(sdiag_i-1)).
"""

import os
import numpy as np

import concourse.bass as bass  # noqa: F401
from concourse import bacc
import concourse.mybir as mybir
import concourse.tile as tile
from concourse.bass_utils import run_bass_kernel_spmd

B, D, M, C = 2048, 128, 16384, 3
NCORES = 8
NTILES = B // 128          # 16 anchor tiles of 128
CH = 512                   # matmul free chunk (one PSUM bank)
W = 2048                   # PSUM block (4 banks) = one ACT call
BASE_TEMP = 0.07

F32 = mybir.dt.float32
BF16 = mybir.dt.bfloat16
AF = mybir.ActivationFunctionType
ALU = mybir.AluOpType
AX = mybir.AxisListType

LAST_EXEC_TIME_NS = None   # set by kernel() when SUPCON_TRACE=1


def _install_trace_shim():
    """Register the NTFF profile hook that this image's antenv lacks."""
    import sys
    import types
    import ctypes
    import contextlib

    try:
        from antenv.axon_hooks import get_axon_ntff_profile_hook  # noqa: F401
        return True
    except ImportError:
        pass

    so_path = "/opt/axon/libaxon_pjrt.so"
    if not os.path.exists(so_path):
        return False
    lib = ctypes.CDLL(so_path)
    if not hasattr(lib, "axon_start_nrt_profile"):
        return False
    lib.axon_start_nrt_profile.argtypes = [
        ctypes.POINTER(ctypes.c_int64),
        ctypes.c_size_t,
    ]
    lib.axon_start_nrt_profile.restype = ctypes.c_int64
    lib.axon_stop_nrt_profile.argtypes = [ctypes.c_char_p]
    lib.axon_stop_nrt_profile.restype = ctypes.c_int64

    @contextlib.contextmanager
    def _hook(output_dir, device_ids):
        import jax

        jax.devices()
        if device_ids:
            ids = (ctypes.c_int64 * len(device_ids))(*device_ids)
            rc = lib.axon_start_nrt_profile(ids, len(device_ids))
        else:
            rc = lib.axon_start_nrt_profile(None, 0)
        if rc != 0:
            raise RuntimeError(f"axon_start_nrt_profile rc={rc}")
        try:
            yield
        finally:
            n = lib.axon_stop_nrt_profile(str(output_dir).encode())
            print(f"profile: {n} file(s) written to {output_dir}", file=sys.stderr)

    _state = {"hook": _hook}
    mod = types.ModuleType("antenv.axon_hooks")
    mod.get_axon_ntff_profile_hook = lambda: _state["hook"]
    mod.set_axon_ntff_profile_hook = lambda h: _state.update(hook=h)
    sys.modules["antenv.axon_hooks"] = mod
    import antenv

    antenv.axon_hooks = mod

    import concourse.bass_utils as bu

    bu.upload_artifacts = lambda tmpdir: tmpdir
    return True


# ----------------------------------------------------------------------
# Host planning
# ----------------------------------------------------------------------

def _make_plan(lab, blab):
    """Compile-time plan from the label histograms (baked into the
    program; identical for all cores)."""
    cnt = np.bincount(lab, minlength=C)
    by_class = [np.where(lab == c)[0] for c in range(C)]
    fb = [(int(cnt[c]) // 128) * 128 for c in range(C)]
    pure_idx = np.concatenate([by_class[c][: fb[c]] for c in range(C)])
    left_idx = np.concatenate([by_class[c][fb[c]:] for c in range(C)])
    bord = np.concatenate([pure_idx, left_idx]).astype(np.int64)
    slab = lab[bord]

    tile_classes = [
        sorted(set(slab[t * 128:(t + 1) * 128].tolist())) for t in range(NTILES)
    ]
    mixed_ids = [t for t in range(NTILES) if len(tile_classes[t]) > 1]
    nm = len(mixed_ids)

    mord = np.argsort(blab, kind="stable").astype(np.int64)
    m = np.bincount(blab, minlength=C).astype(np.int64)
    seg = [0, int(m[0]), int(m[0] + m[1]), M]

    # position p (0..15) -> tile id; mixed tiles must land on odd
    # positions (slot1) of the last cores.
    pure_ids = [t for t in range(NTILES) if t not in mixed_ids]
    tile_of_pos = [None] * NTILES
    mixed_pos = [NTILES - 1 - 2 * i for i in range(nm)]  # 15, 13
    for i, p in enumerate(mixed_pos):
        tile_of_pos[p] = mixed_ids[nm - 1 - i]
    it = iter(pure_ids)
    for p in range(NTILES):
        if tile_of_pos[p] is None:
            tile_of_pos[p] = next(it)

    cmin = int(np.argmin(m))
    gclasses = [c for c in range(C) if c != cmin]        # slot1 group classes
    gsizes = [int(m[c]) for c in gclasses]
    KB = sum(gsizes)                                     # = M - m[cmin]

    # FS: per mixed tile, the cmin-class segment striped over 8 cores
    fs_per = int(-(-int(m[cmin]) // NCORES)) if nm else 0
    fs_runs = [
        {"wslot": 2 + i, "cls": cmin, "per_core": fs_per, "total": int(m[cmin])}
        for i in range(nm)
    ]
    F0 = fs_per * nm

    return {
        "bord": bord, "mord": mord, "slab": slab, "m": m, "seg": seg,
        "cnt": cnt, "tile_of_pos": tile_of_pos, "mixed_ids": mixed_ids,
        "nm": nm, "cmin": cmin, "gclasses": gclasses, "gsizes": gsizes,
        "KB": KB, "fs_runs": fs_runs, "F0": F0,
    }


def _make_stream(plan):
    """The uniform per-core column stream: list of segments
    (region, src_off, length, wslot, cutgroup).  region in
    {'A','B','F','E'} (bankA, bankB, bankF SBUF tiles, emb)."""
    KB = plan["KB"]
    segs = []
    segs.append(("A", 0, KB, 0, "s0"))
    segs.append(("E", 0, B, 0, "s0"))          # slot0 bank+bb share a group
    off = 0
    for c, g in zip(plan["gclasses"], plan["gsizes"]):
        segs.append(("B", off, g, 1, f"s1g{c}"))
        off += g
    segs.append(("E", 0, B, 1, "s1e"))
    foff = 0
    for r in plan["fs_runs"]:
        segs.append(("F", foff, r["per_core"], r["wslot"], f"fs{r['wslot']}"))
        foff += r["per_core"]
    return segs


def _make_blocks(segs):
    """Blocks = ACT calls.  New block at every cutgroup change; within a
    group, 2048-col blocks.  Each block: list of chunks
    (region, src_off, width, wslot) with width<=512, plus call meta."""
    blocks = []
    cur = None

    def flush():
        nonlocal cur
        if cur and cur["width"] > 0:
            blocks.append(cur)
        cur = None

    for (reg, soff, length, ws, grp) in segs:
        pos = 0
        while pos < length:
            if cur is not None and (cur["grp"] != grp or cur["width"] >= W):
                flush()
            if cur is None:
                cur = {"grp": grp, "wslot": ws, "width": 0, "chunks": []}
            take = min(length - pos, W - cur["width"])
            # split into <=512 matmul chunks ALIGNED to PSUM banks: a
            # single matmul output cannot cross a 512-col PSUM bank edge
            cpos = 0
            while cpos < take:
                ppos = cur["width"] + cpos
                cw = min(CH - (ppos % CH), take - cpos)
                cur["chunks"].append((reg, soff + pos + cpos, cw, ws))
                cpos += cw
            cur["width"] += take
            pos += take
            if cur["width"] >= W:
                flush()
    flush()
    return blocks


# ----------------------------------------------------------------------
# Device program
# ----------------------------------------------------------------------

def _build(plan, blocks):
    import ml_dtypes  # noqa: F401

    KB, F0, nm = plan["KB"], plan["F0"], plan["nm"]
    NW = 2 + nm
    NCALLS = len(blocks)
    NOUT = NCALLS + 2 + 6   # accums | sdiag x2 | raw3 x2

    nc = bacc.Bacc()
    embT_d = nc.declare_dram_parameter("embT", [D, B], BF16, isOutput=False)
    anchT_d = nc.declare_dram_parameter("anchT", [D, 128 * NW + C], BF16,
                                        isOutput=False)
    bankA_d = nc.declare_dram_parameter("bankA", [D, KB], BF16, isOutput=False)
    bankB_d = nc.declare_dram_parameter("bankB", [D, KB], BF16, isOutput=False)
    if F0:
        bankF_d = nc.declare_dram_parameter("bankF", [D, F0], BF16,
                                            isOutput=False)
    # vecs: [invt x NW | ninvt x NW | eye128]
    NV = 2 * NW + 128
    vecs_d = nc.declare_dram_parameter("vecs", [128, NV], F32, isOutput=False)
    oout_d = nc.declare_dram_parameter("oout", [128, NOUT], F32, isOutput=True)

    with tile.TileContext(nc) as tc:
        with (
            tc.tile_pool(name="big", bufs=1) as bigp,
            tc.tile_pool(name="sm", bufs=1) as smp,
            tc.tile_pool(name="ps", bufs=2, space="PSUM") as psp,
        ):
            anch_t = bigp.tile([D, 128 * NW + C], BF16, tag="anchT")
            vecs_t = smp.tile([128, NV], F32, tag="vecs")
            junkw_t = bigp.tile([128, 128], BF16, tag="junkw")
            emb_t = bigp.tile([D, B], BF16, tag="embT")
            bankA_t = bigp.tile([D, KB], BF16, tag="bankA")
            bankB_t = bigp.tile([D, KB], BF16, tag="bankB")
            if F0:
                bankF_t = bigp.tile([D, F0], BF16, tag="bankF", name="bankF_t")
            else:
                bankF_t = None
            scr_t = smp.tile([128, W], F32, tag="scr")
            oout_t = smp.tile([128, NOUT], F32, tag="oout")
            eyemul = smp.tile([128, 128], F32, tag="eyemul")
            warm = smp.tile([128, 1], F32, tag="warm")

            invt_t = vecs_t[:, 0:NW]
            ninvt_t = vecs_t[:, NW:2 * NW]
            eye_t = vecs_t[:, 2 * NW:2 * NW + 128]

            regions = {"A": bankA_t, "B": bankB_t, "F": bankF_t, "E": emb_t}

            def anch(ws):
                return anch_t[:, ws * 128:(ws + 1) * 128]

            # ---- ACT table-load warmup FIRST on the scalar engine's
            # program (t ~ 0, before any scalar-queue DMA issue) ----
            nc.vector.memset(junkw_t[:], 0.0)
            nc.scalar.activation(warm[:], junkw_t[:, 0:1], AF.Exp,
                                 bias=0.0, scale=0.0)

            # ---- DMA plan.  The HWDGE sequencer stalls on DIRECT2D when
            # its queue is full (depth ~4), so the scalar queue gets only
            # 4 upfront pieces; later scalar pieces are interleaved
            # between EXP calls (emitted inside the block loop).  The
            # sync engine has no compute, so it takes everything else.
            def pieces(t, d, total, sizes):
                out, pos = [], 0
                for sz in sizes:
                    if pos >= total:
                        return out
                    take = min(sz, total - pos)
                    out.append((t[:, pos:pos + take], d[:, pos:pos + take]))
                    pos += take
                while pos < total:
                    take = min(W, total - pos)
                    out.append((t[:, pos:pos + take], d[:, pos:pos + take]))
                    pos += take
                return out

            A_pc = pieces(bankA_t, bankA_d, KB, [512, 512, 1024])
            E_pc = pieces(emb_t, embT_d, B, [1024, 1024])
            B_pc = pieces(bankB_t, bankB_d, KB, [])
            F_pc = pieces(bankF_t, bankF_d, F0, []) if F0 else []

            # scalar HWDGE: exactly 4 upfront pieces (fits queue depth, so
            # the sequencer never stalls and no DMA issue ever sits
            # between EXP calls); sync HWDGE: the early critical chain;
            # gpsimd SWDGE: everything with big slack.
            scalar_up = [A_pc[i] for i in (1, 3, 5) if i < len(A_pc)]
            if len(A_pc) > 7:
                scalar_up.append(A_pc[7])
            scalar_set = set(id(p[0]) for p in scalar_up)
            sync_all = ([(anch_t[:], anchT_d[:])]
                        + [A_pc[0]]
                        + [(vecs_t[:], vecs_d[:])]
                        + [p for i, p in enumerate(A_pc)
                           if i not in (0, 1, 3, 5, 7)]
                        + E_pc + B_pc[0:2])
            pool_all = B_pc[2:] + F_pc

            for o, i in scalar_up:
                nc.scalar.dma_start(out=o, in_=i)
            for o, i in sync_all:
                nc.sync.dma_start(out=o, in_=i)
            if pool_all:
                # Gate the SWDGE burst behind EXP0's accumulator so its
                # 2.2MB doesn't steal HBM bandwidth from the critical
                # early pieces (per-core HBM is shared by all queues).
                gdum = smp.tile([128, 1], F32, tag="gdum", name="gdum")
                nc.gpsimd.tensor_copy(out=gdum[:], in_=oout_t[:, 0:1])
                for o, i in pool_all:
                    nc.gpsimd.dma_start(out=o, in_=i)

            # ---- prelude: self-sim diags + raw3 (class-sum dots) ----
            pre_ps = psp.tile([128, W], F32, tag="chunk", name="pre_ps")
            for t in range(2):
                nc.tensor.matmul(
                    pre_ps[:, t * 128:(t + 1) * 128], anch(t), anch(t),
                    start=True, stop=True,
                )
                nc.tensor.matmul(
                    pre_ps[:, 256 + t * C:256 + (t + 1) * C], anch(t),
                    anch_t[:, 128 * NW:128 * NW + C], start=True, stop=True,
                )
            for t in range(2):
                nc.vector.tensor_mul(eyemul[:], pre_ps[:, t * 128:(t + 1) * 128],
                                     eye_t[:])
                nc.vector.reduce_sum(oout_t[:, NCALLS + t:NCALLS + t + 1],
                                     eyemul[:], axis=AX.X)
                nc.vector.tensor_copy(
                    out=oout_t[:, NCALLS + 2 + t * C:NCALLS + 2 + (t + 1) * C],
                    in_=pre_ps[:, 256 + t * C:256 + (t + 1) * C])

            # ---- main stream: blocks of matmul chunks + one EXP call ----
            for j, blk in enumerate(blocks):
                ps = psp.tile([128, W], F32, tag="chunk", name=f"blk{j}")
                pos = 0
                for (reg, soff, cw, ws) in blk["chunks"]:
                    nc.tensor.matmul(
                        ps[:, pos:pos + cw], anch(ws),
                        regions[reg][:, soff:soff + cw],
                        start=True, stop=True,
                    )
                    pos += cw
                wsl = blk["wslot"]
                nc.scalar.activation(
                    scr_t[:, 0:blk["width"]], ps[:, 0:blk["width"]], AF.Exp,
                    bias=ninvt_t[:, wsl:wsl + 1], scale=invt_t[:, wsl:wsl + 1],
                    accum_out=oout_t[:, j:j + 1],
                )


            nc.sync.dma_start(out=oout_d[:], in_=oout_t[:])

    nc.compile()
    return nc


# ----------------------------------------------------------------------
# Host packing + reduction
# ----------------------------------------------------------------------

def kernel(embeddings, labels, bank_embs, bank_labels, class_temps):
    global LAST_EXEC_TIME_NS
    import ml_dtypes

    emb = np.asarray(embeddings, dtype=np.float32)
    bank = np.asarray(bank_embs, dtype=np.float32)
    lab = np.asarray(labels).astype(np.int64).ravel()
    blab = np.asarray(bank_labels).astype(np.int64).ravel()
    ct = np.asarray(class_temps, dtype=np.float32).ravel()

    plan = _make_plan(lab, blab)
    segs = _make_stream(plan)
    blocks = _make_blocks(segs)
    NCALLS = len(blocks)
    nm, KB, F0 = plan["nm"], plan["KB"], plan["F0"]
    NW = 2 + nm
    bord, mord, slab = plan["bord"], plan["mord"], plan["slab"]
    seg, cmin = plan["seg"], plan["cmin"]
    tile_of_pos = plan["tile_of_pos"]

    embT = np.ascontiguousarray(emb[bord].T).astype(ml_dtypes.bfloat16)  # [D,B]
    bankT = np.ascontiguousarray(bank[mord].T).astype(ml_dtypes.bfloat16)
    smlab = blab[mord]

    temps = ct[slab]
    inv_t = (1.0 / temps).astype(np.float32)             # [B] sorted order
    cnt = plan["cnt"]
    pos_cnt = cnt[slab] - 1
    n_valid = int((pos_cnt > 0).sum())

    # class-sum embedding vectors (from the same bf16-rounded data)
    gT = np.stack(
        [emb[bord][slab == c].sum(axis=0) for c in range(C)], axis=1
    ).astype(ml_dtypes.bfloat16)                          # [D, 3]

    # per-class bank column index lists (positions in mord order)
    cls_cols = [np.arange(seg[c], seg[c + 1]) for c in range(C)]

    def pure_cols(c):
        return np.concatenate([cls_cols[cc] for cc in range(C) if cc != c])

    # pack one bank slot: returns (bf16 [D, width], tag [width])
    def pack(cols, width):
        out = np.zeros((D, width), dtype=ml_dtypes.bfloat16)
        tags = np.full(width, -2, dtype=np.int64)
        k = len(cols)
        out[:, :k] = bankT[:, cols]
        tags[:k] = smlab[cols]
        return out, tags

    # mixed-tile home slot: group-aligned packing
    def mixed_home():
        outs, tags = [], []
        for c, g in zip(plan["gclasses"], plan["gsizes"]):
            o, t = pack(cls_cols[c], g)
            outs.append(o)
            tags.append(t)
        return np.concatenate(outs, axis=1), np.concatenate(tags)

    in_maps = []
    tagsA, tagsB, tagsF = [], [], []
    core_tiles = []
    for core in range(NCORES):
        t0, t1 = tile_of_pos[2 * core], tile_of_pos[2 * core + 1]
        core_tiles.append((t0, t1))
        wslot_tiles = [t0, t1] + plan["mixed_ids"]

        def tclass(t):
            s = set(slab[t * 128:(t + 1) * 128].tolist())
            return next(iter(s)) if len(s) == 1 else None

        a_cols = pure_cols(tclass(t0))
        bA, tgA = pack(a_cols, KB)
        if t1 in plan["mixed_ids"]:
            bB, tgB = mixed_home()
        else:
            bB, tgB = pack(pure_cols(tclass(t1)), KB)
        tagsA.append(tgA)
        tagsB.append(tgB)

        fs_parts, fs_tags = [], []
        for r in plan["fs_runs"]:
            lo = core * r["per_core"]
            hi = min(lo + r["per_core"], r["total"])
            cols = cls_cols[r["cls"]][lo:hi] if hi > lo else np.array([], int)
            o, t = pack(cols, r["per_core"])
            fs_parts.append(o)
            fs_tags.append(t)
        tagsF.append(np.concatenate(fs_tags) if fs_parts else
                     np.zeros(0, dtype=np.int64))

        anchT = np.zeros((D, 128 * NW + C), dtype=ml_dtypes.bfloat16)
        for w, t in enumerate(wslot_tiles):
            anchT[:, w * 128:(w + 1) * 128] = embT[:, t * 128:(t + 1) * 128]
        anchT[:, 128 * NW:] = gT

        vecs = np.zeros((128, 2 * NW + 128), dtype=np.float32)
        for w, t in enumerate(wslot_tiles):
            vecs[:, w] = inv_t[t * 128:(t + 1) * 128]
            vecs[:, NW + w] = -inv_t[t * 128:(t + 1) * 128]
        vecs[:, 2 * NW:] = np.eye(128, dtype=np.float32)

        im = {
            "embT": embT,
            "anchT": np.ascontiguousarray(anchT),
            "bankA": np.ascontiguousarray(bA),
            "bankB": np.ascontiguousarray(bB),
            "vecs": np.ascontiguousarray(vecs),
        }
        if F0:
            im["bankF"] = np.ascontiguousarray(np.concatenate(fs_parts, axis=1))
        in_maps.append(im)

    nc = _build(plan, blocks)

    trace = os.environ.get("SUPCON_TRACE", "0") == "1"
    if trace:
        trace = _install_trace_shim()
    res = run_bass_kernel_spmd(nc, in_maps, core_ids=list(range(NCORES)),
                               trace=trace)
    LAST_EXEC_TIME_NS = res.exec_time_ns

    # ---- host reduction ----
    # per-core per-call class sets + dummy counts from the tag arrays
    region_tags = {"A": tagsA, "B": tagsB, "F": tagsF}
    den = np.zeros(B, dtype=np.float64)           # sorted-anchor order
    sdiag = np.zeros(B, dtype=np.float64)
    raw3 = np.zeros((B, C), dtype=np.float64)
    einv = np.exp(-inv_t.astype(np.float64))      # exp(-invt_i) per anchor

    accs = []
    for core in range(NCORES):
        oo = np.asarray(res.results[core]["oout"], dtype=np.float64)
        accs.append(oo)
        t0, t1 = core_tiles[core]
        for t, slot in ((t0, 0), (t1, 1)):
            rows = slice(t * 128, (t + 1) * 128)
            sdiag[rows] = oo[:, NCALLS + slot]
            raw3[rows] = oo[:, NCALLS + 2 + slot * C:NCALLS + 2 + (slot + 1) * C]

    for core in range(NCORES):
        oo = accs[core]
        t0, t1 = core_tiles[core]
        wslot_tiles = [t0, t1] + plan["mixed_ids"]
        for j, blk in enumerate(blocks):
            t = wslot_tiles[blk["wslot"]]
            rows = slice(t * 128, (t + 1) * 128)
            lt = slab[rows]
            # gather this call's bank tags (bb chunks have no tags)
            tags = []
            for (reg, soff, cw, _ws) in blk["chunks"]:
                if reg != "E":
                    tags.append(region_tags[reg][core][soff:soff + cw])
            if tags:
                tags = np.concatenate(tags)
                ndum = int((tags == -2).sum())
                cls_set = set(tags[tags >= 0].tolist())
            else:
                ndum, cls_set = 0, set()
            inc = ~np.isin(lt, list(cls_set)) if cls_set else \
                np.ones(128, dtype=bool)
            contrib = oo[:, j] - ndum * einv[rows]
            den[rows] += np.where(inc, contrib, 0.0)

    den -= np.exp(inv_t.astype(np.float64) * (sdiag - 1.0))

    own_raw = raw3[np.arange(B), slab]
    pos_mean_raw = (own_raw - sdiag) / np.maximum(pos_cnt, 1)
    invt64 = inv_t.astype(np.float64)
    coef = (BASE_TEMP / temps).astype(np.float64)
    # loss_i = coef * (invt*(1 - pos_mean_raw) + log(den))
    loss_i = coef * (invt64 * (1.0 - pos_mean_raw) + np.log(den))
    valid = pos_cnt > 0
    loss = np.where(valid, loss_i, 0.0).sum() / max(n_valid, 1)
    return np.float32(loss)
